# revision 9
# baseline (speedup 1.0000x reference)
"""ChannelBlockImportanceGate kernel for 8 Trainium2 NeuronCores.

Computes, per (b, c) slice of features [8, 256, 132, 132] f32:
  scores = block-sum of |x| over 8x8 blocks (17x17 grid, zero-padded edges)
  top-72 blocks (ties -> lowest index, matching jax.lax.top_k)
  output = per-pixel {0,1} mask upsampled 8x8 (cropped to 132x132)

The straight-through soft term of the reference cancels in the forward
pass (hard - sg(soft) + soft == hard up to ~1ulp), so the output is the
hard mask.

Sharding: purely data parallel. 2048 (b,c) slices -> 256 per core.
Per core: 2 groups of 128 slices; each slice occupies one SBUF
partition so pooling/topk/upsample are per-partition ops with no
cross-partition traffic. Top-72 uses 9 rounds of DVE max8 +
match_replace(-1e30), then mask = (score < 0).

v2 design (from perfetto analysis of v1):
 - The 16 SDMA engines each need ~83us busy (load f32 + store f32
   sides both run at the ~27GB/s per-engine port rate; cast-stores
   do NOT reduce engine busy time -- measured). v1's DMA_15 ran ~24%
   slower than its peers (101.5us busy, zero idle -> it WAS the
   critical path). The suspected cause is GpSimd Q7 SBUF-port
   contention from the 3.7us gpsimd tensor_copies; v2 has NO gpsimd
   compute at all.
 - The mask is materialized PACKED: one f32 element = 4 fp8(0|1)
   pixels (mask * 0x38383838-as-f32, exact since mask is {0,1}).
   Upsample copies move 4x fewer elements (4.8us/group on one
   engine instead of 19us), done by scalar+vector, freeing gpsimd.
 - Stores are SWDGE (nc.gpsimd.dma_start) with fp8->f32 cast into
   the f32 output (bit-exact, verified). They ride the Pool queue,
   separate from the sync-queue loads, so engines round-robin
   load/store packets instead of FIFOing through one queue.
 - Vector chain: pool g0 (18.7us) -> topk g0 (9.5us) -> pool g1 ->
   topk g1, finishing ~70us so the g1 store stream (last ~21us of
   per-engine DMA work) is descriptor-fed in time.
"""

import numpy as np

B, C, H, W = 8, 256, 132, 132
HW = H * W            # 17424
NB = 17               # 8x8 blocks per side (132 padded to 136)
NBLK = NB * NB        # 289
KEEP = 72             # round(289 * 0.25)
N_CORES = 8
S = (B * C) // N_CORES  # 256 slices per core
WP = W // 4           # 33 packed u32 per pixel row
LOAD_CHUNKS = ((0, 16), (16, 48), (48, 80), (80, 104), (104, 120),
               (120, 132))
STORE_CHUNKS = ((0, 16), (16, 48), (48, 80), (80, 104), (104, 120),
                (120, 132))
NEG = -1.0e30
# f32 whose 4 bytes are each fp8e4m3(1.0) = 0x38; mask * PACK4 produces
# the packed 4-pixel fp8 row exactly (mask is exactly 0.0 or 1.0).
PACK4 = float(np.frombuffer(np.uint32(0x38383838).tobytes(),
                            dtype=np.float32)[0])

_prog_cache = {}


def _build_program():
    import concourse.bacc as bacc
    import concourse.mybir as mybir
    import concourse.tile as tile

    f32 = mybir.dt.float32
    fp8 = mybir.dt.float8e4
    X = mybir.AxisListType.X
    XY = mybir.AxisListType.XY
    ADD = mybir.AluOpType.add

    nc = bacc.Bacc("TRN2", debug=False, num_devices=N_CORES)
    x = nc.dram_tensor("x", (S, HW), f32, kind="ExternalInput")
    y = nc.dram_tensor("y", (S, HW), f32, kind="ExternalOutput")

    with tile.TileContext(nc) as tc:
        with (
            tc.tile_pool(name="big", bufs=2) as bigp,
            tc.tile_pool(name="med", bufs=2) as medp,
            tc.tile_pool(name="small", bufs=2) as smallp,
        ):
            # All load DMAs are emitted before any store DMA. Loads ride
            # the sync HWDGE queue (g0's first chunk on the scalar HWDGE
            # queue, measured fastest in v1); stores ride the Pool
            # (SWDGE) queue so load/store packets interleave round-robin
            # on each SDMA engine.
            xb = []
            li = 0
            for g in range(S // 128):
                p0 = g * 128
                xt = bigp.tile([128, HW], f32, name=f"xb_g{g}", tag="xb")
                for k, (r0, r1) in enumerate(LOAD_CHUNKS):
                    # Alternate the two HWDGE queues: per-DMA completion
                    # overhead serializes within a queue but overlaps
                    # across queues (mb2 vs mb4: 117us -> 97us for the
                    # same 24 chunked DMAs).
                    eng = nc.sync if li % 2 == 0 else nc.scalar
                    li += 1
                    eng.dma_start(out=xt[:, r0 * W:r1 * W],
                                  in_=x[p0:p0 + 128, r0 * W:r1 * W])
                xb.append(xt)

            for g in range(S // 128):
                p0 = g * 128
                xt = xb[g]
                xv = xt.rearrange("p (r w) -> p r w", w=W)

                # Fused 8x8 block pooling: one XY tensor_reduce per chunk
                # computes scores[p, h, q] = sum |x| over the full 8x8
                # block directly from the raw pixels. Edge strips are 3
                # small XY reduces.
                scores = smallp.tile([128, NBLK], f32,
                                     name=f"scores_g{g}", tag="scores")
                sc3 = scores.rearrange("p (h t) -> p h t", t=NB)
                if g > 0:
                    # Ordering token (from v1): pins this group's pooling
                    # after the previous group's mask on the vector
                    # engine, otherwise the scheduler interleaves the two
                    # groups' pooling and delays the first mask by ~20us.
                    nc.vector.tensor_copy(
                        out=scores[0:1, :],
                        in_=prev_pm[0:1, 0:1].broadcast_to((1, NBLK)))
                for k, (r0, r1) in enumerate(LOAD_CHUNKS):
                    rr1 = min(r1, 128)
                    nc.vector.tensor_reduce(
                        out=sc3[:, r0 // 8:rr1 // 8, 0:16],
                        in_=(xv[:, r0:rr1, 0:128]
                             .rearrange("p (h r) (q c) -> p h q r c",
                                        r=8, c=8)),
                        axis=XY, op=ADD, apply_absolute_value=True)
                nc.vector.tensor_reduce(
                    out=sc3[:, 0:16, 16:17],
                    in_=(xv[:, 0:128, 128:132]
                         .rearrange("p (h r) c -> p h r c", r=8)),
                    axis=XY, op=ADD, apply_absolute_value=True)
                nc.vector.tensor_reduce(
                    out=sc3[:, 16:17, 0:16],
                    in_=(xv[:, 128:132, 0:128]
                         .rearrange("p r (q c) -> p q r c", c=8)),
                    axis=XY, op=ADD, apply_absolute_value=True)
                nc.vector.tensor_reduce(
                    out=sc3[:, 16:17, 16:17],
                    in_=xv[:, 128:132, 128:132].unsqueeze(1),
                    axis=XY, op=ADD, apply_absolute_value=True)

                # Top-72 per partition: 9 rounds of max8 + match_replace.
                # match_replace replaces the first unmatched occurrence, so
                # ties resolve to the lowest index like jax.lax.top_k.
                for it in range(KEEP // 8):
                    m8 = smallp.tile([128, 8], f32,
                                     name=f"m8_g{g}i{it}", tag="m8")
                    nc.vector.max(out=m8[:, :], in_=scores[:, :])
                    nc.vector.match_replace(out=scores[:, :],
                                            in_to_replace=m8[:, :],
                                            in_values=scores[:, :],
                                            imm_value=NEG)

                # Packed block mask: replaced entries are -1e30, so
                # (score < 0) * PACK4 writes 0x38383838 (4 fp8 ones) for
                # selected blocks, 0.0 for the rest. pm[p, h*17+q].
                pm = smallp.tile([128, NBLK], f32, name=f"pm_g{g}",
                                 tag="pm")
                nc.vector.tensor_scalar(out=pm[:, :], in0=scores[:, :],
                                        scalar1=0.0, scalar2=PACK4,
                                        op0=mybir.AluOpType.is_lt,
                                        op1=mybir.AluOpType.mult)
                pm3 = pm.rearrange("p (h t) -> p h t", t=NB)
                prev_pm = pm

                # Packed row-mask [p, h, 33]: one 132-px row (33 packed
                # elems) per block-row; blocks 0..15 span 2 packed elems
                # each, edge block 16 spans exactly 1 (pixels 128-131).
                rm = medp.tile([128, NB * WP], f32, name=f"rm_g{g}",
                               tag="rm")
                rm3 = rm.rearrange("p (h w) -> p h w", w=WP)
                nc.vector.tensor_copy(
                    out=rm3[:, :, 0:32].rearrange("p h (q c) -> p h q c",
                                                  c=2),
                    in_=(pm3[:, :, 0:16].unsqueeze(3)
                         .broadcast_to((128, NB, 16, 2))))
                nc.vector.tensor_copy(
                    out=rm3[:, :, 32:33],
                    in_=pm3[:, :, 16:17])

                # Vertical 8x upsample into the packed mask tile, then
                # SWDGE cast-store (fp8 -> f32) per chunk. Upsample is 4x
                # fewer elements than v1 (packed), split scalar/vector.
                mk = medp.tile([128, NB * 8 * WP], f32, name=f"mk_g{g}",
                               tag="mk")
                mk4 = mk.rearrange("p (h r w) -> p h r w", r=8, w=WP)
                if g == 0:
                    # Hold g0's stores until ALL loads have landed: after
                    # the last load (t_L ~ 53us) there are still ~41us of
                    # store DMA work, so the tail is store-bound and
                    # starting g0's stores early only steals load
                    # bandwidth and delays g1's topk. Each 1-elem token
                    # copy reads g1's last load chunk (RAW) and writes the
                    # first element of one store chunk's mk region (WAW
                    # with that chunk's upsample copy), pinning every g0
                    # upsample copy - and hence store - after the loads.
                    for (r0, r1) in STORE_CHUNKS:
                        nc.scalar.copy(
                            out=mk[0:1, r0 * WP:r0 * WP + 1],
                            in_=xb[1][0:1, HW - 1:HW])
                for k, (r0, r1) in enumerate(STORE_CHUNKS):
                    h0, h1 = r0 // 8, (r1 + 7) // 8
                    nr = min(r1, 136) - r0
                    src = (rm3[:, h0:h1, :].unsqueeze(2)
                           .broadcast_to((128, h1 - h0, 8, WP)))
                    dst = mk4[:, h0:h1, :, :]
                    nc.scalar.copy(out=dst, in_=src)
                    # Store rows r0:r1 (crop block-row 16 to 4 rows via
                    # the flat view of mk).
                    nc.gpsimd.dma_start(
                        out=y[p0:p0 + 128, r0 * W:r1 * W],
                        in_=mk[:, r0 * WP:r1 * WP].bitcast(fp8))
    nc.compile()
    return nc


def _ensure_ntff_hook_module():
    """bass_utils' trace path does `from antenv.axon_hooks import
    get_axon_ntff_profile_hook` — a module this image doesn't ship.
    Register an equivalent (ctypes into libaxon_pjrt.so, mirroring
    trn_boot._ntff_profile_via_ctypes) so BASS_TRACE=1 works; degrade
    to a None hook (trace skipped) when unavailable."""
    import sys
    import types

    try:
        import antenv.axon_hooks  # noqa: F401
        return
    except Exception:
        pass

    hook = None
    try:
        import contextlib
        import ctypes

        so_path = "/opt/axon/libaxon_pjrt.so"
        lib = ctypes.CDLL(so_path)
        if hasattr(lib, "axon_start_nrt_profile"):
            lib.axon_start_nrt_profile.argtypes = [
                ctypes.POINTER(ctypes.c_int64), ctypes.c_size_t]
            lib.axon_start_nrt_profile.restype = ctypes.c_int64
            lib.axon_stop_nrt_profile.argtypes = [ctypes.c_char_p]
            lib.axon_stop_nrt_profile.restype = ctypes.c_int64

            @contextlib.contextmanager
            def _hook(output_dir, device_ids):
                import jax
                jax.devices()
                if device_ids:
                    ids = (ctypes.c_int64 * len(device_ids))(*device_ids)
                    rc = lib.axon_start_nrt_profile(ids, len(device_ids))
                else:
                    rc = lib.axon_start_nrt_profile(None, 0)
                if rc != 0:
                    raise RuntimeError(f"axon_start_nrt_profile rc={rc}")
                try:
                    yield
                finally:
                    n = lib.axon_stop_nrt_profile(str(output_dir).encode())
                    print(f"ntff profile: {n} file(s) -> {output_dir}",
                          file=sys.stderr)

            hook = _hook
    except Exception:
        hook = None

    mod = types.ModuleType("antenv.axon_hooks")
    mod.get_axon_ntff_profile_hook = lambda: hook
    mod.set_axon_ntff_profile_hook = lambda h: None
    sys.modules["antenv.axon_hooks"] = mod


def _get_program():
    if "nc" not in _prog_cache:
        _prog_cache["nc"] = _build_program()
    return _prog_cache["nc"]


def kernel(features, enabled):
    feats = np.asarray(features)
    if not bool(np.asarray(enabled)):
        return np.ones(feats.shape, dtype=np.float32)

    _ensure_ntff_hook_module()
    import concourse.bass_utils as _bu
    from concourse.bass_utils import run_bass_kernel_spmd

    # The trace path uploads artifacts to a shared bucket; tolerate
    # sandboxes where that fails.
    if not getattr(_bu, "_upload_patched", False):
        _orig_upload = _bu.upload_artifacts

        def _safe_upload(tmpdir):
            try:
                return _orig_upload(tmpdir)
            except Exception:
                return str(tmpdir)

        _bu.upload_artifacts = _safe_upload
        _bu._upload_patched = True

    nc = _get_program()
    flat = np.ascontiguousarray(feats.reshape(B * C, HW), dtype=np.float32)
    in_maps = [{"x": flat[i * S:(i + 1) * S]} for i in range(N_CORES)]
    res = run_bass_kernel_spmd(nc, in_maps, list(range(N_CORES)))
    _prog_cache["last_res"] = res
    out = np.concatenate([np.asarray(res.results[i]["y"])
                          for i in range(N_CORES)], axis=0)
    return out.reshape(B, C, H, W).astype(np.float32)


# revision 10
# speedup vs baseline: 1.0018x; 1.0018x over previous
"""ChannelBlockImportanceGate kernel for 8 Trainium2 NeuronCores.

Computes, per (b, c) slice of features [8, 256, 132, 132] f32:
  scores = block-sum of |x| over 8x8 blocks (17x17 grid, zero-padded edges)
  top-72 blocks (ties -> lowest index, matching jax.lax.top_k)
  output = per-pixel {0,1} mask upsampled 8x8 (cropped to 132x132)

The straight-through soft term of the reference cancels in the forward
pass (hard - sg(soft) + soft == hard up to ~1ulp), so the output is the
hard mask.

Sharding: purely data parallel. 2048 (b,c) slices -> 256 per core.
Per core: 2 groups of 128 slices; each slice occupies one SBUF
partition so pooling/topk/upsample are per-partition ops with no
cross-partition traffic. Top-72 uses 9 rounds of DVE max8 +
match_replace(-1e30), then mask = (score < 0).

v2 design (from perfetto analysis of v1):
 - The 16 SDMA engines each need ~83us busy (load f32 + store f32
   sides both run at the ~27GB/s per-engine port rate; cast-stores
   do NOT reduce engine busy time -- measured). v1's DMA_15 ran ~24%
   slower than its peers (101.5us busy, zero idle -> it WAS the
   critical path). The suspected cause is GpSimd Q7 SBUF-port
   contention from the 3.7us gpsimd tensor_copies; v2 has NO gpsimd
   compute at all.
 - The mask is materialized PACKED: one f32 element = 4 fp8(0|1)
   pixels (mask * 0x38383838-as-f32, exact since mask is {0,1}).
   Upsample copies move 4x fewer elements (4.8us/group on one
   engine instead of 19us), done by scalar+vector, freeing gpsimd.
 - Stores are SWDGE (nc.gpsimd.dma_start) with fp8->f32 cast into
   the f32 output (bit-exact, verified). They ride the Pool queue,
   separate from the sync-queue loads, so engines round-robin
   load/store packets instead of FIFOing through one queue.
 - Vector chain: pool g0 (18.7us) -> topk g0 (9.5us) -> pool g1 ->
   topk g1, finishing ~70us so the g1 store stream (last ~21us of
   per-engine DMA work) is descriptor-fed in time.
"""

import numpy as np

B, C, H, W = 8, 256, 132, 132
HW = H * W            # 17424
NB = 17               # 8x8 blocks per side (132 padded to 136)
NBLK = NB * NB        # 289
KEEP = 72             # round(289 * 0.25)
N_CORES = 8
S = (B * C) // N_CORES  # 256 slices per core
WP = W // 4           # 33 packed u32 per pixel row
LOAD_CHUNKS = ((0, 16), (16, 48), (48, 80), (80, 104), (104, 120),
               (120, 132))
STORE_CHUNKS = ((0, 16), (16, 48), (48, 80), (80, 104), (104, 120),
                (120, 132))
NEG = -1.0e30
# f32 whose 4 bytes are each fp8e4m3(1.0) = 0x38; mask * PACK4 produces
# the packed 4-pixel fp8 row exactly (mask is exactly 0.0 or 1.0).
PACK4 = float(np.frombuffer(np.uint32(0x38383838).tobytes(),
                            dtype=np.float32)[0])

_prog_cache = {}


def _build_program():
    import concourse.bacc as bacc
    import concourse.mybir as mybir
    import concourse.tile as tile

    f32 = mybir.dt.float32
    fp8 = mybir.dt.float8e4
    X = mybir.AxisListType.X
    XY = mybir.AxisListType.XY
    ADD = mybir.AluOpType.add

    nc = bacc.Bacc("TRN2", debug=False, num_devices=N_CORES)
    x = nc.dram_tensor("x", (S, HW), f32, kind="ExternalInput")
    y = nc.dram_tensor("y", (S, HW), f32, kind="ExternalOutput")

    with tile.TileContext(nc) as tc:
        with (
            tc.tile_pool(name="big", bufs=2) as bigp,
            tc.tile_pool(name="med", bufs=2) as medp,
            tc.tile_pool(name="small", bufs=2) as smallp,
        ):
            # All load DMAs are emitted before any store DMA. Loads ride
            # the sync HWDGE queue (g0's first chunk on the scalar HWDGE
            # queue, measured fastest in v1); stores ride the Pool
            # (SWDGE) queue so load/store packets interleave round-robin
            # on each SDMA engine.
            xb = []
            li = 0
            for g in range(S // 128):
                p0 = g * 128
                xt = bigp.tile([128, HW], f32, name=f"xb_g{g}", tag="xb")
                for k, (r0, r1) in enumerate(LOAD_CHUNKS):
                    # Alternate the two HWDGE queues: per-DMA completion
                    # overhead serializes within a queue but overlaps
                    # across queues (mb2 vs mb4: 117us -> 97us for the
                    # same 24 chunked DMAs).
                    eng = nc.sync if li % 2 == 0 else nc.scalar
                    li += 1
                    eng.dma_start(out=xt[:, r0 * W:r1 * W],
                                  in_=x[p0:p0 + 128, r0 * W:r1 * W])
                xb.append(xt)

            xv = [xt.rearrange("p (r w) -> p r w", w=W) for xt in xb]
            scores = [smallp.tile([128, NBLK], f32, name=f"scores_g{g}",
                                  tag="scores") for g in range(2)]
            sc3 = [s.rearrange("p (h t) -> p h t", t=NB) for s in scores]

            def pool_chunk(g, r0, r1):
                rr1 = min(r1, 128)
                nc.vector.tensor_reduce(
                    out=sc3[g][:, r0 // 8:rr1 // 8, 0:16],
                    in_=(xv[g][:, r0:rr1, 0:128]
                         .rearrange("p (h r) (q c) -> p h q r c",
                                    r=8, c=8)),
                    axis=XY, op=ADD, apply_absolute_value=True)

            def pool_edges(g):
                nc.vector.tensor_reduce(
                    out=sc3[g][:, 0:16, 16:17],
                    in_=(xv[g][:, 0:128, 128:132]
                         .rearrange("p (h r) c -> p h r c", r=8)),
                    axis=XY, op=ADD, apply_absolute_value=True)
                nc.vector.tensor_reduce(
                    out=sc3[g][:, 16:17, 0:16],
                    in_=(xv[g][:, 128:132, 0:128]
                         .rearrange("p r (q c) -> p q r c", c=8)),
                    axis=XY, op=ADD, apply_absolute_value=True)
                nc.vector.tensor_reduce(
                    out=sc3[g][:, 16:17, 16:17],
                    in_=xv[g][:, 128:132, 128:132].unsqueeze(1),
                    axis=XY, op=ADD, apply_absolute_value=True)

            def topk(g, first_m8):
                # Top-72 per partition: 9 rounds of max8 + match_replace.
                # match_replace replaces the first unmatched occurrence,
                # so ties resolve to the lowest index like jax.lax.top_k.
                for it in range(KEEP // 8):
                    m8 = first_m8 if it == 0 else smallp.tile(
                        [128, 8], f32, name=f"m8_g{g}i{it}", tag="m8")
                    nc.vector.max(out=m8[:, :], in_=scores[g][:, :])
                    nc.vector.match_replace(out=scores[g][:, :],
                                            in_to_replace=m8[:, :],
                                            in_values=scores[g][:, :],
                                            imm_value=NEG)

            def pm_rm(g):
                # Packed block mask: replaced entries are -1e30, so
                # (score < 0) * PACK4 writes 0x38383838 (4 fp8 ones) for
                # selected blocks, 0.0 for the rest. Then the packed
                # row-mask [p, h, 33]: one 132-px row (33 packed elems)
                # per block-row; blocks 0..15 span 2 packed elems each,
                # edge block 16 exactly 1 (pixels 128-131).
                pm = smallp.tile([128, NBLK], f32, name=f"pm_g{g}",
                                 tag="pm")
                nc.vector.tensor_scalar(out=pm[:, :], in0=scores[g][:, :],
                                        scalar1=0.0, scalar2=PACK4,
                                        op0=mybir.AluOpType.is_lt,
                                        op1=mybir.AluOpType.mult)
                pm3 = pm.rearrange("p (h t) -> p h t", t=NB)
                rm = medp.tile([128, NB * WP], f32, name=f"rm_g{g}",
                               tag="rm")
                rm3 = rm.rearrange("p (h w) -> p h w", w=WP)
                nc.vector.tensor_copy(
                    out=rm3[:, :, 0:32].rearrange("p h (q c) -> p h q c",
                                                  c=2),
                    in_=(pm3[:, :, 0:16].unsqueeze(3)
                         .broadcast_to((128, NB, 16, 2))))
                nc.vector.tensor_copy(
                    out=rm3[:, :, 32:33],
                    in_=pm3[:, :, 16:17])
                return pm, rm3

            def upsample_store(g, rm3, hold_src):
                # Vertical 8x upsample into the packed mask tile (scalar),
                # then SWDGE cast-store (fp8 -> f32) per chunk.
                p0 = g * 128
                mk = medp.tile([128, NB * 8 * WP], f32, name=f"mk_g{g}",
                               tag="mk")
                mk4 = mk.rearrange("p (h r w) -> p h r w", r=8, w=WP)
                if hold_src is not None:
                    # Hold this group's stores until all loads have
                    # landed: the tail is store-bound (after the last
                    # load there are still ~41us of store work), so
                    # starting stores early only steals load bandwidth
                    # and delays g1's topk. Each 1-elem token copy reads
                    # g1's second-to-last load chunk (RAW) and writes the
                    # first element of one store chunk's mk region (WAW
                    # with that chunk's upsample copy).
                    for (r0, r1) in STORE_CHUNKS:
                        nc.scalar.copy(
                            out=mk[0:1, r0 * WP:r0 * WP + 1],
                            in_=hold_src)
                for k, (r0, r1) in enumerate(STORE_CHUNKS):
                    h0, h1 = r0 // 8, (r1 + 7) // 8
                    src = (rm3[:, h0:h1, :].unsqueeze(2)
                           .broadcast_to((128, h1 - h0, 8, WP)))
                    nc.scalar.copy(out=mk4[:, h0:h1, :, :], in_=src)
                    # Store rows r0:r1 (crop block-row 16 to 4 rows via
                    # the flat view of mk).
                    nc.gpsimd.dma_start(
                        out=y[p0:p0 + 128, r0 * W:r1 * W],
                        in_=mk[:, r0 * WP:r1 * WP].bitcast(fp8))

            # Vector-chain order (the tail is store-bound; g0's stores
            # are held to t_L anyway, so topk g0 is NOT urgent -- what
            # matters is topk g1 finishing ASAP after the last load):
            #   pool g0 (arrival-paced) -> pool g1 chunk0 -> topk g0 ->
            #   pm/rm g0 -> pool g1 rest -> topk g1 -> pm/rm g1
            for (r0, r1) in LOAD_CHUNKS:
                pool_chunk(0, r0, r1)
            pool_edges(0)
            # g1's first chunk pools between pool g0 and topk g0.
            pool_chunk(1, *LOAD_CHUNKS[0])
            # Pin topk g0 after pool-g1-chunk0: the token reads chunk0's
            # scores (RAW) and writes into the first m8 tile (WAW with
            # the first max8, which overwrites it fully).
            m8_first = smallp.tile([128, 8], f32, name="m8_g0i0",
                                   tag="m8")
            nc.vector.tensor_copy(out=m8_first[0:1, 0:1],
                                  in_=sc3[1][0:1, 0:1, 0:1])
            topk(0, m8_first)
            pm0, rm3_0 = pm_rm(0)
            # Ordering token: pins g1's remaining pooling (block-rows
            # >= 2, flat indices 34:289) after g0's mask, otherwise the
            # scheduler interleaves the groups' pooling and delays g0's
            # mask by ~20us. Chunk0 (block-rows 0-1) stays exempt.
            nc.vector.tensor_copy(
                out=scores[1][0:1, 2 * NB:NBLK],
                in_=pm0[0:1, 0:1].broadcast_to((1, NBLK - 2 * NB)))
            for (r0, r1) in LOAD_CHUNKS[1:]:
                pool_chunk(1, r0, r1)
            pool_edges(1)
            m8_g1 = smallp.tile([128, 8], f32, name="m8_g1i0", tag="m8")
            topk(1, m8_g1)
            pm1, rm3_1 = pm_rm(1)

            hold = xb[1][0:1, LOAD_CHUNKS[-2][0] * W + 1:
                         LOAD_CHUNKS[-2][0] * W + 2]
            upsample_store(0, rm3_0, hold)
            upsample_store(1, rm3_1, None)
    nc.compile()
    return nc


def _ensure_ntff_hook_module():
    """bass_utils' trace path does `from antenv.axon_hooks import
    get_axon_ntff_profile_hook` — a module this image doesn't ship.
    Register an equivalent (ctypes into libaxon_pjrt.so, mirroring
    trn_boot._ntff_profile_via_ctypes) so BASS_TRACE=1 works; degrade
    to a None hook (trace skipped) when unavailable."""
    import sys
    import types

    try:
        import antenv.axon_hooks  # noqa: F401
        return
    except Exception:
        pass

    hook = None
    try:
        import contextlib
        import ctypes

        so_path = "/opt/axon/libaxon_pjrt.so"
        lib = ctypes.CDLL(so_path)
        if hasattr(lib, "axon_start_nrt_profile"):
            lib.axon_start_nrt_profile.argtypes = [
                ctypes.POINTER(ctypes.c_int64), ctypes.c_size_t]
            lib.axon_start_nrt_profile.restype = ctypes.c_int64
            lib.axon_stop_nrt_profile.argtypes = [ctypes.c_char_p]
            lib.axon_stop_nrt_profile.restype = ctypes.c_int64

            @contextlib.contextmanager
            def _hook(output_dir, device_ids):
                import jax
                jax.devices()
                if device_ids:
                    ids = (ctypes.c_int64 * len(device_ids))(*device_ids)
                    rc = lib.axon_start_nrt_profile(ids, len(device_ids))
                else:
                    rc = lib.axon_start_nrt_profile(None, 0)
                if rc != 0:
                    raise RuntimeError(f"axon_start_nrt_profile rc={rc}")
                try:
                    yield
                finally:
                    n = lib.axon_stop_nrt_profile(str(output_dir).encode())
                    print(f"ntff profile: {n} file(s) -> {output_dir}",
                          file=sys.stderr)

            hook = _hook
    except Exception:
        hook = None

    mod = types.ModuleType("antenv.axon_hooks")
    mod.get_axon_ntff_profile_hook = lambda: hook
    mod.set_axon_ntff_profile_hook = lambda h: None
    sys.modules["antenv.axon_hooks"] = mod


def _get_program():
    if "nc" not in _prog_cache:
        _prog_cache["nc"] = _build_program()
    return _prog_cache["nc"]


def kernel(features, enabled):
    feats = np.asarray(features)
    if not bool(np.asarray(enabled)):
        return np.ones(feats.shape, dtype=np.float32)

    _ensure_ntff_hook_module()
    import concourse.bass_utils as _bu
    from concourse.bass_utils import run_bass_kernel_spmd

    # The trace path uploads artifacts to a shared bucket; tolerate
    # sandboxes where that fails.
    if not getattr(_bu, "_upload_patched", False):
        _orig_upload = _bu.upload_artifacts

        def _safe_upload(tmpdir):
            try:
                return _orig_upload(tmpdir)
            except Exception:
                return str(tmpdir)

        _bu.upload_artifacts = _safe_upload
        _bu._upload_patched = True

    nc = _get_program()
    flat = np.ascontiguousarray(feats.reshape(B * C, HW), dtype=np.float32)
    in_maps = [{"x": flat[i * S:(i + 1) * S]} for i in range(N_CORES)]
    res = run_bass_kernel_spmd(nc, in_maps, list(range(N_CORES)))
    _prog_cache["last_res"] = res
    out = np.concatenate([np.asarray(res.results[i]["y"])
                          for i in range(N_CORES)], axis=0)
    return out.reshape(B, C, H, W).astype(np.float32)


# revision 11
# speedup vs baseline: 1.0057x; 1.0039x over previous
"""ChannelBlockImportanceGate kernel for 8 Trainium2 NeuronCores.

Computes, per (b, c) slice of features [8, 256, 132, 132] f32:
  scores = block-sum of |x| over 8x8 blocks (17x17 grid, zero-padded edges)
  top-72 blocks (ties -> lowest index, matching jax.lax.top_k)
  output = per-pixel {0,1} mask upsampled 8x8 (cropped to 132x132)

The straight-through soft term of the reference cancels in the forward
pass (hard - sg(soft) + soft == hard up to ~1ulp), so the output is the
hard mask.

Sharding: purely data parallel. 2048 (b,c) slices -> 256 per core.
Per core: 2 groups of 128 slices; each slice occupies one SBUF
partition so pooling/topk/upsample are per-partition ops with no
cross-partition traffic. Top-72 uses 9 rounds of DVE max8 +
match_replace(-1e30), then mask = (score < 0).

v2 design (from perfetto analysis of v1):
 - The 16 SDMA engines each need ~83us busy (load f32 + store f32
   sides both run at the ~27GB/s per-engine port rate; cast-stores
   do NOT reduce engine busy time -- measured). v1's DMA_15 ran ~24%
   slower than its peers (101.5us busy, zero idle -> it WAS the
   critical path). The suspected cause is GpSimd Q7 SBUF-port
   contention from the 3.7us gpsimd tensor_copies; v2 has NO gpsimd
   compute at all.
 - The mask is materialized PACKED: one f32 element = 4 fp8(0|1)
   pixels (mask * 0x38383838-as-f32, exact since mask is {0,1}).
   Upsample copies move 4x fewer elements (4.8us/group on one
   engine instead of 19us), done by scalar+vector, freeing gpsimd.
 - Stores are SWDGE (nc.gpsimd.dma_start) with fp8->f32 cast into
   the f32 output (bit-exact, verified). They ride the Pool queue,
   separate from the sync-queue loads, so engines round-robin
   load/store packets instead of FIFOing through one queue.
 - Vector chain: pool g0 (18.7us) -> topk g0 (9.5us) -> pool g1 ->
   topk g1, finishing ~70us so the g1 store stream (last ~21us of
   per-engine DMA work) is descriptor-fed in time.
"""

import numpy as np

B, C, H, W = 8, 256, 132, 132
HW = H * W            # 17424
NB = 17               # 8x8 blocks per side (132 padded to 136)
NBLK = NB * NB        # 289
KEEP = 72             # round(289 * 0.25)
N_CORES = 8
S = (B * C) // N_CORES  # 256 slices per core
WP = W // 4           # 33 packed u32 per pixel row
LOAD_CHUNKS = ((0, 16), (16, 48), (48, 80), (80, 104), (104, 120),
               (120, 132))
STORE_CHUNKS = ((0, 16), (16, 48), (48, 80), (80, 104), (104, 120),
                (120, 132))
NEG = -1.0e30
# f32 whose 4 bytes are each fp8e4m3(1.0) = 0x38; mask * PACK4 produces
# the packed 4-pixel fp8 row exactly (mask is exactly 0.0 or 1.0).
PACK4 = float(np.frombuffer(np.uint32(0x38383838).tobytes(),
                            dtype=np.float32)[0])

_prog_cache = {}


def _build_program():
    import concourse.bacc as bacc
    import concourse.mybir as mybir
    import concourse.tile as tile

    f32 = mybir.dt.float32
    fp8 = mybir.dt.float8e4
    X = mybir.AxisListType.X
    XY = mybir.AxisListType.XY
    ADD = mybir.AluOpType.add

    nc = bacc.Bacc("TRN2", debug=False, num_devices=N_CORES)
    x = nc.dram_tensor("x", (S, HW), f32, kind="ExternalInput")
    y = nc.dram_tensor("y", (S, HW), f32, kind="ExternalOutput")

    with tile.TileContext(nc) as tc:
        with (
            tc.tile_pool(name="big", bufs=2) as bigp,
            tc.tile_pool(name="med", bufs=2) as medp,
            tc.tile_pool(name="small", bufs=2) as smallp,
        ):
            # All load DMAs are emitted before any store DMA. Loads ride
            # the sync HWDGE queue (g0's first chunk on the scalar HWDGE
            # queue, measured fastest in v1); stores ride the Pool
            # (SWDGE) queue so load/store packets interleave round-robin
            # on each SDMA engine.
            xb = []
            li = 0
            for g in range(S // 128):
                p0 = g * 128
                xt = bigp.tile([128, HW], f32, name=f"xb_g{g}", tag="xb")
                for k, (r0, r1) in enumerate(LOAD_CHUNKS):
                    # Alternate the two HWDGE queues: per-DMA completion
                    # overhead serializes within a queue but overlaps
                    # across queues (mb2 vs mb4: 117us -> 97us for the
                    # same 24 chunked DMAs).
                    eng = nc.sync if li % 2 == 0 else nc.scalar
                    li += 1
                    eng.dma_start(out=xt[:, r0 * W:r1 * W],
                                  in_=x[p0:p0 + 128, r0 * W:r1 * W])
                xb.append(xt)

            xv = [xt.rearrange("p (r w) -> p r w", w=W) for xt in xb]
            scores = [smallp.tile([128, NBLK], f32, name=f"scores_g{g}",
                                  tag="scores") for g in range(2)]
            sc3 = [s.rearrange("p (h t) -> p h t", t=NB) for s in scores]

            def pool_chunk(g, r0, r1):
                rr1 = min(r1, 128)
                nc.vector.tensor_reduce(
                    out=sc3[g][:, r0 // 8:rr1 // 8, 0:16],
                    in_=(xv[g][:, r0:rr1, 0:128]
                         .rearrange("p (h r) (q c) -> p h q r c",
                                    r=8, c=8)),
                    axis=XY, op=ADD, apply_absolute_value=True)

            def pool_edges(g):
                nc.vector.tensor_reduce(
                    out=sc3[g][:, 0:16, 16:17],
                    in_=(xv[g][:, 0:128, 128:132]
                         .rearrange("p (h r) c -> p h r c", r=8)),
                    axis=XY, op=ADD, apply_absolute_value=True)
                nc.vector.tensor_reduce(
                    out=sc3[g][:, 16:17, 0:16],
                    in_=(xv[g][:, 128:132, 0:128]
                         .rearrange("p r (q c) -> p q r c", c=8)),
                    axis=XY, op=ADD, apply_absolute_value=True)
                nc.vector.tensor_reduce(
                    out=sc3[g][:, 16:17, 16:17],
                    in_=xv[g][:, 128:132, 128:132].unsqueeze(1),
                    axis=XY, op=ADD, apply_absolute_value=True)

            def topk(g, first_m8):
                # Top-72 per partition: 9 rounds of max8 + match_replace.
                # match_replace replaces the first unmatched occurrence,
                # so ties resolve to the lowest index like jax.lax.top_k.
                for it in range(KEEP // 8):
                    m8 = first_m8 if it == 0 else smallp.tile(
                        [128, 8], f32, name=f"m8_g{g}i{it}", tag="m8")
                    nc.vector.max(out=m8[:, :], in_=scores[g][:, :])
                    nc.vector.match_replace(out=scores[g][:, :],
                                            in_to_replace=m8[:, :],
                                            in_values=scores[g][:, :],
                                            imm_value=NEG)

            def pm_rm(g):
                # Packed block mask: replaced entries are -1e30, so
                # (score < 0) * PACK4 writes 0x38383838 (4 fp8 ones) for
                # selected blocks, 0.0 for the rest. Then the packed
                # row-mask [p, h, 33]: one 132-px row (33 packed elems)
                # per block-row; blocks 0..15 span 2 packed elems each,
                # edge block 16 exactly 1 (pixels 128-131).
                pm = smallp.tile([128, NBLK], f32, name=f"pm_g{g}",
                                 tag="pm")
                nc.vector.tensor_scalar(out=pm[:, :], in0=scores[g][:, :],
                                        scalar1=0.0, scalar2=PACK4,
                                        op0=mybir.AluOpType.is_lt,
                                        op1=mybir.AluOpType.mult)
                pm3 = pm.rearrange("p (h t) -> p h t", t=NB)
                rm = medp.tile([128, NB * WP], f32, name=f"rm_g{g}",
                               tag="rm")
                rm3 = rm.rearrange("p (h w) -> p h w", w=WP)
                nc.vector.tensor_copy(
                    out=rm3[:, :, 0:32].rearrange("p h (q c) -> p h q c",
                                                  c=2),
                    in_=(pm3[:, :, 0:16].unsqueeze(3)
                         .broadcast_to((128, NB, 16, 2))))
                nc.vector.tensor_copy(
                    out=rm3[:, :, 32:33],
                    in_=pm3[:, :, 16:17])
                return pm, rm3

            def upsample_store(g, rm3, hold_src):
                # Vertical 8x upsample into the packed mask tile (scalar),
                # then SWDGE cast-store (fp8 -> f32) per chunk.
                p0 = g * 128
                mk = medp.tile([128, NB * 8 * WP], f32, name=f"mk_g{g}",
                               tag="mk")
                mk4 = mk.rearrange("p (h r w) -> p h r w", r=8, w=WP)
                if hold_src is not None:
                    # Hold back this group's LAST TWO store chunks until
                    # g1's mask (pm1) exists: g0's stores otherwise drain
                    # ~4us before g1's stores can start (topk g1 + mask +
                    # first upsample + SWDGE emission), leaving the DMA
                    # engines idle in that window. Pinning ~3us of g0
                    # store work behind pm1 keeps the pipe fed through
                    # the handoff. 1-elem token copies: read pm1 (RAW),
                    # write the held chunks' first mk elements (WAW with
                    # their upsample copies).
                    for (r0, r1) in STORE_CHUNKS[-2:]:
                        nc.scalar.copy(
                            out=mk[0:1, r0 * WP:r0 * WP + 1],
                            in_=hold_src[0:1, 0:1])
                for k, (r0, r1) in enumerate(STORE_CHUNKS):
                    h0, h1 = r0 // 8, (r1 + 7) // 8
                    src = (rm3[:, h0:h1, :].unsqueeze(2)
                           .broadcast_to((128, h1 - h0, 8, WP)))
                    nc.scalar.copy(out=mk4[:, h0:h1, :, :], in_=src)
                    # Store rows r0:r1 (crop block-row 16 to 4 rows via
                    # the flat view of mk).
                    nc.gpsimd.dma_start(
                        out=y[p0:p0 + 128, r0 * W:r1 * W],
                        in_=mk[:, r0 * WP:r1 * WP].bitcast(fp8))

            # Vector-chain order (the tail is store-bound; g0's stores
            # are held to t_L anyway, so topk g0 is NOT urgent -- what
            # matters is topk g1 finishing ASAP after the last load):
            #   pool g0 (arrival-paced) -> pool g1 chunk0 -> topk g0 ->
            #   pm/rm g0 -> pool g1 rest -> topk g1 -> pm/rm g1
            for (r0, r1) in LOAD_CHUNKS:
                pool_chunk(0, r0, r1)
            pool_edges(0)
            m8_g0 = smallp.tile([128, 8], f32, name="m8_g0i0", tag="m8")
            topk(0, m8_g0)
            pm0, rm3_0 = pm_rm(0)
            # Ordering token: pins g1's pooling after g0's mask on the
            # vector engine, otherwise the scheduler interleaves the two
            # groups' pooling and delays g0's mask by ~20us.
            nc.vector.tensor_copy(
                out=scores[1][0:1, :],
                in_=pm0[0:1, 0:1].broadcast_to((1, NBLK)))
            for (r0, r1) in LOAD_CHUNKS:
                pool_chunk(1, r0, r1)
            pool_edges(1)
            m8_g1 = smallp.tile([128, 8], f32, name="m8_g1i0", tag="m8")
            topk(1, m8_g1)
            pm1, rm3_1 = pm_rm(1)

            upsample_store(0, rm3_0, pm1)
            upsample_store(1, rm3_1, None)
    nc.compile()
    return nc


def _ensure_ntff_hook_module():
    """bass_utils' trace path does `from antenv.axon_hooks import
    get_axon_ntff_profile_hook` — a module this image doesn't ship.
    Register an equivalent (ctypes into libaxon_pjrt.so, mirroring
    trn_boot._ntff_profile_via_ctypes) so BASS_TRACE=1 works; degrade
    to a None hook (trace skipped) when unavailable."""
    import sys
    import types

    try:
        import antenv.axon_hooks  # noqa: F401
        return
    except Exception:
        pass

    hook = None
    try:
        import contextlib
        import ctypes

        so_path = "/opt/axon/libaxon_pjrt.so"
        lib = ctypes.CDLL(so_path)
        if hasattr(lib, "axon_start_nrt_profile"):
            lib.axon_start_nrt_profile.argtypes = [
                ctypes.POINTER(ctypes.c_int64), ctypes.c_size_t]
            lib.axon_start_nrt_profile.restype = ctypes.c_int64
            lib.axon_stop_nrt_profile.argtypes = [ctypes.c_char_p]
            lib.axon_stop_nrt_profile.restype = ctypes.c_int64

            @contextlib.contextmanager
            def _hook(output_dir, device_ids):
                import jax
                jax.devices()
                if device_ids:
                    ids = (ctypes.c_int64 * len(device_ids))(*device_ids)
                    rc = lib.axon_start_nrt_profile(ids, len(device_ids))
                else:
                    rc = lib.axon_start_nrt_profile(None, 0)
                if rc != 0:
                    raise RuntimeError(f"axon_start_nrt_profile rc={rc}")
                try:
                    yield
                finally:
                    n = lib.axon_stop_nrt_profile(str(output_dir).encode())
                    print(f"ntff profile: {n} file(s) -> {output_dir}",
                          file=sys.stderr)

            hook = _hook
    except Exception:
        hook = None

    mod = types.ModuleType("antenv.axon_hooks")
    mod.get_axon_ntff_profile_hook = lambda: hook
    mod.set_axon_ntff_profile_hook = lambda h: None
    sys.modules["antenv.axon_hooks"] = mod


def _get_program():
    if "nc" not in _prog_cache:
        _prog_cache["nc"] = _build_program()
    return _prog_cache["nc"]


def kernel(features, enabled):
    feats = np.asarray(features)
    if not bool(np.asarray(enabled)):
        return np.ones(feats.shape, dtype=np.float32)

    _ensure_ntff_hook_module()
    import concourse.bass_utils as _bu
    from concourse.bass_utils import run_bass_kernel_spmd

    # The trace path uploads artifacts to a shared bucket; tolerate
    # sandboxes where that fails.
    if not getattr(_bu, "_upload_patched", False):
        _orig_upload = _bu.upload_artifacts

        def _safe_upload(tmpdir):
            try:
                return _orig_upload(tmpdir)
            except Exception:
                return str(tmpdir)

        _bu.upload_artifacts = _safe_upload
        _bu._upload_patched = True

    nc = _get_program()
    flat = np.ascontiguousarray(feats.reshape(B * C, HW), dtype=np.float32)
    in_maps = [{"x": flat[i * S:(i + 1) * S]} for i in range(N_CORES)]
    res = run_bass_kernel_spmd(nc, in_maps, list(range(N_CORES)))
    _prog_cache["last_res"] = res
    out = np.concatenate([np.asarray(res.results[i]["y"])
                          for i in range(N_CORES)], axis=0)
    return out.reshape(B, C, H, W).astype(np.float32)


# revision 14
# speedup vs baseline: 1.1391x; 1.1326x over previous
"""ChannelBlockImportanceGate kernel for 8 Trainium2 NeuronCores.

Computes, per (b, c) slice of features [8, 256, 132, 132] f32:
  scores = block-sum of |x| over 8x8 blocks (17x17 grid, zero-padded edges)
  top-72 blocks (ties -> lowest index, matching jax.lax.top_k)
  output = per-pixel {0,1} mask upsampled 8x8 (cropped to 132x132)

The straight-through soft term of the reference cancels in the forward
pass (hard - sg(soft) + soft == hard up to ~1ulp), so the output is the
hard mask.

Sharding: purely data parallel. 2048 (b,c) slices -> 256 per core.
Per core: 2 groups of 128 slices; each slice occupies one SBUF
partition so pooling/topk/upsample are per-partition ops with no
cross-partition traffic. Top-72 uses 9 rounds of DVE max8 +
match_replace(-1e30), then mask = (score < 0).

Performance model (from extensive perfetto analysis + microbenches):
 - The kernel is DMA-bound: 17.8MB load + 17.8MB store per core at
   the ~26.4GB/s per-SDMA-engine SBUF-port rate = ~84.5us of busy
   time on each of the 16 engines. A pure DMA round-trip of the same
   bytes measures 96-118us depending on an ENVIRONMENTAL mode (SDMA
   engine 15 sporadically runs ~20% slower, minutes-timescale; not
   controllable from the kernel). The full kernel measures within
   +-noise of that pure-DMA control, i.e. all compute is hidden.
 - Cast-during-DMA does NOT reduce engine busy time (engine time is
   set by the f32 side -- measured), but the PACKED fp8 mask makes
   the upsample 4x cheaper on the compute engines: one f32 element =
   4 fp8(0|1) pixels (mask * 0x38383838-as-f32, exact since mask is
   {0,1}); SWDGE stores cast fp8->f32 bit-exactly into the output.
 - DMAs are spread across all three queues (loads alternate the two
   HWDGE queues, stores ride the Pool/SWDGE queue): per-DMA
   completion overhead serializes within a queue but overlaps across
   queues (measured 117us -> 97us for the same 24 chunked DMAs).
 - Vector chain: pool g0 (18.7us, arrival-paced chunks) -> topk g0
   (9.5us) -> pool g1 -> topk g1, finishing ~77us so the g1 store
   stream (the last ~21us of per-engine DMA work) starts ~79us and
   the kernel lands ~104us in the clean mode, ~110-118 in the slow
   environmental mode.
 - GpSimd does NO compute (only SWDGE store emission); scalar does
   all upsample copies; vector does pooling/topk/masks.
"""

import numpy as np

B, C, H, W = 8, 256, 132, 132
HW = H * W            # 17424
NB = 17               # 8x8 blocks per side (132 padded to 136)
NBLK = NB * NB        # 289
KEEP = 72             # round(289 * 0.25)
N_CORES = 8
S = (B * C) // N_CORES  # 256 slices per core
WP = W // 4           # 33 packed u32 per pixel row
LOAD_CHUNKS = ((0, 16), (16, 48), (48, 80), (80, 104), (104, 120),
               (120, 132))
STORE_CHUNKS = ((0, 16), (16, 48), (48, 80), (80, 104), (104, 120),
                (120, 132))
NEG = -1.0e30
# f32 whose 4 bytes are each fp8e4m3(1.0) = 0x38; mask * PACK4 produces
# the packed 4-pixel fp8 row exactly (mask is exactly 0.0 or 1.0).
PACK4 = float(np.frombuffer(np.uint32(0x38383838).tobytes(),
                            dtype=np.float32)[0])

_prog_cache = {}


def _build_program():
    import concourse.bacc as bacc
    import concourse.mybir as mybir
    import concourse.tile as tile

    f32 = mybir.dt.float32
    fp8 = mybir.dt.float8e4
    X = mybir.AxisListType.X
    XY = mybir.AxisListType.XY
    ADD = mybir.AluOpType.add

    nc = bacc.Bacc("TRN2", debug=False, num_devices=N_CORES)
    x = nc.dram_tensor("x", (S, HW), f32, kind="ExternalInput")
    y = nc.dram_tensor("y", (S, HW), f32, kind="ExternalOutput")

    with tile.TileContext(nc) as tc:
        with (
            tc.tile_pool(name="big", bufs=2) as bigp,
            tc.tile_pool(name="med", bufs=2) as medp,
            tc.tile_pool(name="small", bufs=2) as smallp,
        ):
            # All load DMAs are emitted before any store DMA. Loads ride
            # the sync HWDGE queue (g0's first chunk on the scalar HWDGE
            # queue, measured fastest in v1); stores ride the Pool
            # (SWDGE) queue so load/store packets interleave round-robin
            # on each SDMA engine.
            xb = []
            li = 0
            for g in range(S // 128):
                p0 = g * 128
                xt = bigp.tile([128, HW], f32, name=f"xb_g{g}", tag="xb")
                for k, (r0, r1) in enumerate(LOAD_CHUNKS):
                    # Alternate the two HWDGE queues: per-DMA completion
                    # overhead serializes within a queue but overlaps
                    # across queues (mb2 vs mb4: 117us -> 97us for the
                    # same 24 chunked DMAs).
                    eng = nc.sync if li % 2 == 0 else nc.scalar
                    li += 1
                    eng.dma_start(out=xt[:, r0 * W:r1 * W],
                                  in_=x[p0:p0 + 128, r0 * W:r1 * W])
                xb.append(xt)

            xv = [xt.rearrange("p (r w) -> p r w", w=W) for xt in xb]
            scores = [smallp.tile([128, NBLK], f32, name=f"scores_g{g}",
                                  tag="scores") for g in range(2)]
            sc3 = [s.rearrange("p (h t) -> p h t", t=NB) for s in scores]

            def pool_chunk(g, r0, r1):
                rr1 = min(r1, 128)
                nc.vector.tensor_reduce(
                    out=sc3[g][:, r0 // 8:rr1 // 8, 0:16],
                    in_=(xv[g][:, r0:rr1, 0:128]
                         .rearrange("p (h r) (q c) -> p h q r c",
                                    r=8, c=8)),
                    axis=XY, op=ADD, apply_absolute_value=True)

            def pool_edges(g):
                nc.vector.tensor_reduce(
                    out=sc3[g][:, 0:16, 16:17],
                    in_=(xv[g][:, 0:128, 128:132]
                         .rearrange("p (h r) c -> p h r c", r=8)),
                    axis=XY, op=ADD, apply_absolute_value=True)
                nc.vector.tensor_reduce(
                    out=sc3[g][:, 16:17, 0:16],
                    in_=(xv[g][:, 128:132, 0:128]
                         .rearrange("p r (q c) -> p q r c", c=8)),
                    axis=XY, op=ADD, apply_absolute_value=True)
                nc.vector.tensor_reduce(
                    out=sc3[g][:, 16:17, 16:17],
                    in_=xv[g][:, 128:132, 128:132].unsqueeze(1),
                    axis=XY, op=ADD, apply_absolute_value=True)

            def topk(g, first_m8):
                # Top-72 per partition: 9 rounds of max8 + match_replace.
                # match_replace replaces the first unmatched occurrence,
                # so ties resolve to the lowest index like jax.lax.top_k.
                for it in range(KEEP // 8):
                    m8 = first_m8 if it == 0 else smallp.tile(
                        [128, 8], f32, name=f"m8_g{g}i{it}", tag="m8")
                    nc.vector.max(out=m8[:, :], in_=scores[g][:, :])
                    nc.vector.match_replace(out=scores[g][:, :],
                                            in_to_replace=m8[:, :],
                                            in_values=scores[g][:, :],
                                            imm_value=NEG)

            def pm_rm(g):
                # Packed block mask: replaced entries are -1e30, so
                # (score < 0) * PACK4 writes 0x38383838 (4 fp8 ones) for
                # selected blocks, 0.0 for the rest. Then the packed
                # row-mask [p, h, 33]: one 132-px row (33 packed elems)
                # per block-row; blocks 0..15 span 2 packed elems each,
                # edge block 16 exactly 1 (pixels 128-131).
                pm = smallp.tile([128, NBLK], f32, name=f"pm_g{g}",
                                 tag="pm")
                nc.vector.tensor_scalar(out=pm[:, :], in0=scores[g][:, :],
                                        scalar1=0.0, scalar2=PACK4,
                                        op0=mybir.AluOpType.is_lt,
                                        op1=mybir.AluOpType.mult)
                pm3 = pm.rearrange("p (h t) -> p h t", t=NB)
                rm = medp.tile([128, NB * WP], f32, name=f"rm_g{g}",
                               tag="rm")
                rm3 = rm.rearrange("p (h w) -> p h w", w=WP)
                nc.vector.tensor_copy(
                    out=rm3[:, :, 0:32].rearrange("p h (q c) -> p h q c",
                                                  c=2),
                    in_=(pm3[:, :, 0:16].unsqueeze(3)
                         .broadcast_to((128, NB, 16, 2))))
                nc.vector.tensor_copy(
                    out=rm3[:, :, 32:33],
                    in_=pm3[:, :, 16:17])
                return pm, rm3

            def upsample_store(g, rm3):
                # Vertical 8x upsample into the packed mask tile (scalar),
                # then SWDGE cast-store (fp8 -> f32) per chunk.
                p0 = g * 128
                mk = medp.tile([128, NB * 8 * WP], f32, name=f"mk_g{g}",
                               tag="mk")
                mk4 = mk.rearrange("p (h r w) -> p h r w", r=8, w=WP)
                for k, (r0, r1) in enumerate(STORE_CHUNKS):
                    h0, h1 = r0 // 8, (r1 + 7) // 8
                    src = (rm3[:, h0:h1, :].unsqueeze(2)
                           .broadcast_to((128, h1 - h0, 8, WP)))
                    nc.scalar.copy(out=mk4[:, h0:h1, :, :], in_=src)
                    # Store rows r0:r1 (crop block-row 16 to 4 rows via
                    # the flat view of mk).
                    nc.gpsimd.dma_start(
                        out=y[p0:p0 + 128, r0 * W:r1 * W],
                        in_=mk[:, r0 * WP:r1 * WP].bitcast(fp8))

            # Vector-chain order (the tail is store-bound; g0's stores
            # are held to t_L anyway, so topk g0 is NOT urgent -- what
            # matters is topk g1 finishing ASAP after the last load):
            #   pool g0 (arrival-paced) -> pool g1 chunk0 -> topk g0 ->
            #   pm/rm g0 -> pool g1 rest -> topk g1 -> pm/rm g1
            for (r0, r1) in LOAD_CHUNKS:
                pool_chunk(0, r0, r1)
            pool_edges(0)
            m8_g0 = smallp.tile([128, 8], f32, name="m8_g0i0", tag="m8")
            topk(0, m8_g0)
            pm0, rm3_0 = pm_rm(0)
            # Ordering token: pins g1's pooling after g0's mask on the
            # vector engine, otherwise the scheduler interleaves the two
            # groups' pooling and delays g0's mask by ~20us.
            nc.vector.tensor_copy(
                out=scores[1][0:1, :],
                in_=pm0[0:1, 0:1].broadcast_to((1, NBLK)))
            for (r0, r1) in LOAD_CHUNKS:
                pool_chunk(1, r0, r1)
            pool_edges(1)
            m8_g1 = smallp.tile([128, 8], f32, name="m8_g1i0", tag="m8")
            topk(1, m8_g1)
            pm1, rm3_1 = pm_rm(1)

            upsample_store(0, rm3_0)
            upsample_store(1, rm3_1)
    nc.compile()
    return nc


def _ensure_ntff_hook_module():
    """bass_utils' trace path does `from antenv.axon_hooks import
    get_axon_ntff_profile_hook` — a module this image doesn't ship.
    Register an equivalent (ctypes into libaxon_pjrt.so, mirroring
    trn_boot._ntff_profile_via_ctypes) so BASS_TRACE=1 works; degrade
    to a None hook (trace skipped) when unavailable."""
    import sys
    import types

    try:
        import antenv.axon_hooks  # noqa: F401
        return
    except Exception:
        pass

    hook = None
    try:
        import contextlib
        import ctypes

        so_path = "/opt/axon/libaxon_pjrt.so"
        lib = ctypes.CDLL(so_path)
        if hasattr(lib, "axon_start_nrt_profile"):
            lib.axon_start_nrt_profile.argtypes = [
                ctypes.POINTER(ctypes.c_int64), ctypes.c_size_t]
            lib.axon_start_nrt_profile.restype = ctypes.c_int64
            lib.axon_stop_nrt_profile.argtypes = [ctypes.c_char_p]
            lib.axon_stop_nrt_profile.restype = ctypes.c_int64

            @contextlib.contextmanager
            def _hook(output_dir, device_ids):
                import jax
                jax.devices()
                if device_ids:
                    ids = (ctypes.c_int64 * len(device_ids))(*device_ids)
                    rc = lib.axon_start_nrt_profile(ids, len(device_ids))
                else:
                    rc = lib.axon_start_nrt_profile(None, 0)
                if rc != 0:
                    raise RuntimeError(f"axon_start_nrt_profile rc={rc}")
                try:
                    yield
                finally:
                    n = lib.axon_stop_nrt_profile(str(output_dir).encode())
                    print(f"ntff profile: {n} file(s) -> {output_dir}",
                          file=sys.stderr)

            hook = _hook
    except Exception:
        hook = None

    mod = types.ModuleType("antenv.axon_hooks")
    mod.get_axon_ntff_profile_hook = lambda: hook
    mod.set_axon_ntff_profile_hook = lambda h: None
    sys.modules["antenv.axon_hooks"] = mod


def _get_program():
    if "nc" not in _prog_cache:
        _prog_cache["nc"] = _build_program()
    return _prog_cache["nc"]


def kernel(features, enabled):
    feats = np.asarray(features)
    if not bool(np.asarray(enabled)):
        return np.ones(feats.shape, dtype=np.float32)

    _ensure_ntff_hook_module()
    import concourse.bass_utils as _bu
    from concourse.bass_utils import run_bass_kernel_spmd

    # The trace path uploads artifacts to a shared bucket; tolerate
    # sandboxes where that fails.
    if not getattr(_bu, "_upload_patched", False):
        _orig_upload = _bu.upload_artifacts

        def _safe_upload(tmpdir):
            try:
                return _orig_upload(tmpdir)
            except Exception:
                return str(tmpdir)

        _bu.upload_artifacts = _safe_upload
        _bu._upload_patched = True

    nc = _get_program()
    flat = np.ascontiguousarray(feats.reshape(B * C, HW), dtype=np.float32)
    in_maps = [{"x": flat[i * S:(i + 1) * S]} for i in range(N_CORES)]
    res = run_bass_kernel_spmd(nc, in_maps, list(range(N_CORES)))
    _prog_cache["last_res"] = res
    out = np.concatenate([np.asarray(res.results[i]["y"])
                          for i in range(N_CORES)], axis=0)
    return out.reshape(B, C, H, W).astype(np.float32)
